# revision 2
# baseline (speedup 1.0000x reference)
"""Trainium2 Bass kernel for nn_MaxYager2d.

Math: out[b,f,sh,sw] = max_j relu(1 - (a_j + b_jf)^(1/p))
  with a_j = (1-xu_j)^p (unfold window values), b_jf = (1-w_jf)^p, p=1.5.
Since 1-(t)^(1/p) is monotone decreasing in t, the max commutes to a min:
  out = relu(1 - (min_j (a_j + b_jf))^(1/p))
i.e. a 3x3 min-plus convolution (tropical conv) + pointwise pow.

Sharding: 8 cores = 4 batches x 2 halves of F (16 channels each).
On-chip layout: 128 partitions = (ci in 0..7) x (fi in 0..15), where
channel c = co*8 + ci (co in 0..3 lives on the free axis). A is
replicated across the 16 fi partitions so that b[(c,kh,kw),f] is a
per-partition scalar, letting one fused scalar_tensor_tensor
(acc = min(A_shifted + b, acc)) per (co,kh,kw) do all the work.
A final 3-step partition-halving TT-min reduces over ci.
"""

import numpy as np

C = 32
K = 3
H = 66          # input spatial
S = 64          # output spatial
L = S * S
J = C * K * K   # 288
F = 32
B = 4
CI = 8          # channels on partitions
CO = 4          # channels on free axis
FH = 16         # f channels per core
HH = H * H      # 4356
NCORES = 8

_cache = {}


def _build_program():
    import concourse.tile as tile
    from concourse import bacc, mybir

    f32 = mybir.dt.float32
    Alu = mybir.AluOpType
    Act = mybir.ActivationFunctionType

    nc = bacc.Bacc("TRN2", target_bir_lowering=False, debug=False,
                   num_devices=NCORES)

    x_rep = nc.dram_tensor("x_rep", [128, CO * HH], f32,
                           kind="ExternalInput").ap()
    w_sc = nc.dram_tensor("w_sc", [128, CO * K * K], f32,
                          kind="ExternalInput").ap()
    out_d = nc.dram_tensor("out", [FH, L], f32, kind="ExternalOutput").ap()

    with tile.TileContext(nc) as tc:
        with tc.tile_pool(name="big", bufs=1) as big, \
             tc.tile_pool(name="small", bufs=1) as small:
            # ---- load x (pre-replicated across fi partitions) ----
            xr = big.tile([128, CO * HH], f32)
            nc.sync.dma_start(xr[:], x_rep)

            # A = (1-x)^1.5 = y*sqrt(y), y = 1-x   (in place in xr)
            nc.vector.tensor_scalar(xr[:], xr[:], -1.0, 1.0,
                                    Alu.mult, Alu.add)
            sq = big.tile([128, CO * HH], f32)
            nc.scalar.activation(sq[:], xr[:], Act.Sqrt)
            nc.vector.tensor_tensor(xr[:], xr[:], sq[:], Alu.mult)

            # ---- b = (1-w)^1.5 on [128, 36] ----
            wt = small.tile([128, CO * K * K], f32)
            nc.sync.dma_start(wt[:], w_sc)
            nc.vector.tensor_scalar(wt[:], wt[:], -1.0, 1.0,
                                    Alu.mult, Alu.add)
            sqw = small.tile([128, CO * K * K], f32)
            nc.scalar.activation(sqw[:], wt[:], Act.Sqrt)
            nc.vector.tensor_tensor(wt[:], wt[:], sqw[:], Alu.mult)

            # ---- tropical conv: acc = min over (co,kh,kw) of A_shift + b ----
            acc = big.tile([128, L], f32)
            A4 = xr[:].rearrange("p (co h w) -> p co h w", co=CO, h=H, w=H)
            acc3 = acc[:].rearrange("p (h w) -> p h w", h=S, w=S)
            first = True
            for co in range(CO):
                for kh in range(K):
                    for kw in range(K):
                        av = A4[:, co, kh:kh + S, kw:kw + S]
                        bcol = wt[:, co * 9 + kh * 3 + kw:
                                  co * 9 + kh * 3 + kw + 1]
                        if first:
                            nc.vector.tensor_scalar(acc3, av, bcol, None,
                                                    Alu.add)
                            first = False
                        else:
                            nc.vector.scalar_tensor_tensor(
                                acc3, av, bcol, acc3, Alu.add, Alu.min)

            # ---- reduce over ci: 3 partition-halving min steps ----
            # TT requires equal base partitions, so DMA-realign the upper
            # half down to partition 0 before each min.
            tmp = big.tile([64, L], f32)
            for hi, n in ((64, 64), (32, 32), (16, 16)):
                nc.sync.dma_start(tmp[0:n], acc[hi:hi + n])
                nc.vector.tensor_tensor(acc[0:n], acc[0:n], tmp[0:n],
                                        Alu.min)

            # ---- out = relu(1 - m^(2/3)) via exp((2/3) ln m) ----
            res = small.tile([FH, L], f32)
            nc.scalar.activation(res[:], acc[0:16], Act.Ln)
            nc.scalar.activation(res[:], res[:], Act.Exp, scale=2.0 / 3.0)
            nc.scalar.activation(res[:], res[:], Act.Relu,
                                 bias=1.0, scale=-1.0)

            nc.sync.dma_start(out_d, res[:])

    nc.compile()
    return nc


def _get_nc():
    if "nc" not in _cache:
        _cache["nc"] = _build_program()
    return _cache["nc"]


def _shard_inputs(x, weight):
    """Host-side sharding/layout. Returns in_maps for 8 cores."""
    in_maps = []
    for core in range(NCORES):
        b = core // 2
        fb = (core % 2) * FH
        xb = np.asarray(x[b], dtype=np.float32).reshape(C, HH)
        # [co, ci, s] -> [ci, co, s], replicate over fi -> [ci, fi, co, s]
        t = xb.reshape(CO, CI, HH).transpose(1, 0, 2)
        xrep = np.broadcast_to(t[:, None, :, :], (CI, FH, CO, HH))
        xrep = np.ascontiguousarray(xrep).reshape(128, CO * HH)

        wsl = np.asarray(weight[:, fb:fb + FH], dtype=np.float32)
        # [c, khw, fi] -> [co, ci, khw, fi] -> [ci, fi, co, khw]
        r = wsl.reshape(CO, CI, K * K, FH).transpose(1, 3, 0, 2)
        wsc = np.ascontiguousarray(r).reshape(128, CO * K * K)

        in_maps.append({"x_rep": xrep, "w_sc": wsc})
    return in_maps


def kernel(x, weight):
    from concourse.bass_utils import run_bass_kernel_spmd

    nc = _get_nc()
    in_maps = _shard_inputs(x, weight)
    res = run_bass_kernel_spmd(nc, in_maps, list(range(NCORES)))
    out = np.empty((B, F, S, S), dtype=np.float32)
    for core in range(NCORES):
        b = core // 2
        fb = (core % 2) * FH
        out[b, fb:fb + FH] = res.results[core]["out"].reshape(FH, S, S)
    return out


# revision 3
# speedup vs baseline: 1.8083x; 1.8083x over previous
"""Trainium2 Bass kernel for nn_MaxYager2d.

Math: out[b,f,sh,sw] = max_j relu(1 - (a_j + b_jf)^(1/p))
  with a_j = (1-xu_j)^p (3x3 unfold windows), b_jf = (1-w_jf)^p, p=1.5.
Since 1-t^(1/p) is monotone decreasing, the max commutes to a min:
  out = relu(1 - (min_j (a_j + b_jf))^(1/p))
i.e. a 3x3 min-plus (tropical) convolution + pointwise pow.

Sharding: 8 cores = 4 batches x 2 halves of F (16 out-channels each).

On-chip layout: 128 partitions = (ci in 0..7) x (fi in 0..15); input
channel c = co*8 + ci with co in 0..3 on the free axis. A = (1-x)^1.5 is
computed compactly, bounced through an HBM scratch, and broadcast-read
16x across the fi partitions (plus a byte-shifted copy so every bf16
slice AP is 4B-aligned). Then per (co,kh,kw): tmp = A_shift + b (4x-mode
tensor_scalar on DVE, or Activation with per-partition bias on ScalarE),
acc = min(acc, tmp) (2x-mode bf16 TT on DVE). A 3-step DMA-realign +
TT-min folds the remaining ci axis, and a short ACT chain applies
relu(1 - m^(2/3)) = relu(1 - exp((2/3) ln m)).
"""

import numpy as np

C = 32
K = 3
H = 66          # input spatial
S = 64          # output spatial
L = S * S
F = 32
B = 4
CI = 8          # channels on partitions
CO = 4          # channels on free axis
FH = 16         # f channels per core
HH = H * H      # 4356
XROWS = 128
XCOLS = C * HH // XROWS  # 1089
NCORES = 8

_cache = {}


def _build_program():
    import concourse.tile as tile
    from concourse import bacc, mybir

    f32 = mybir.dt.float32
    bf16 = mybir.dt.bfloat16
    Alu = mybir.AluOpType
    Act = mybir.ActivationFunctionType

    nc = bacc.Bacc("TRN2", target_bir_lowering=False, debug=False,
                   num_devices=NCORES)

    x_c = nc.dram_tensor("x_c", [XROWS, XCOLS], f32,
                         kind="ExternalInput").ap()
    w_sc = nc.dram_tensor("w_sc", [128, CO * K * K], f32,
                          kind="ExternalInput").ap()
    out_d = nc.dram_tensor("out", [FH, L], f32, kind="ExternalOutput").ap()
    a_hbm = nc.dram_tensor("a_scratch", [C * HH], bf16)

    with tile.TileContext(nc) as tc:
        with tc.tile_pool(name="big", bufs=1) as big, \
             tc.tile_pool(name="small", bufs=1) as small, \
             tc.tile_pool(name="tmp", bufs=6) as tmppool:
            # ---- A = (1-x)^1.5 computed compactly ----
            xt = small.tile([XROWS, XCOLS], f32)
            nc.sync.dma_start(xt[:], x_c)
            nc.vector.tensor_scalar(xt[:], xt[:], -1.0, 1.0,
                                    Alu.mult, Alu.add)       # y = 1-x
            sq = small.tile([XROWS, XCOLS], f32)
            nc.scalar.activation(sq[:], xt[:], Act.Sqrt)
            a_c = small.tile([XROWS, XCOLS], bf16)
            nc.vector.tensor_tensor(a_c[:], xt[:], sq[:], Alu.mult)

            # bounce through HBM, broadcast-read 16x across fi partitions
            nc.sync.dma_start(a_hbm.ap(), a_c[:])
            av_ch = a_hbm.ap().rearrange("(c s) -> c s", c=C)  # [32, 4356]
            a_rep = big.tile([128, CO * HH], bf16)
            a_odd = big.tile([128, CO * HH], bf16)
            for co in range(CO):
                blk = av_ch[co * CI:(co + 1) * CI]
                src = blk.unsqueeze(1).broadcast_to([CI, FH, HH])
                nc.sync.dma_start(
                    a_rep[:, co * HH:(co + 1) * HH], src)
                # byte-shifted copy so kw=1 slices stay 4B-aligned
                blk1 = av_ch[co * CI:(co + 1) * CI, 1:HH]
                src1 = blk1.unsqueeze(1).broadcast_to([CI, FH, HH - 1])
                nc.sync.dma_start(
                    a_odd[:, co * HH:co * HH + HH - 1], src1)

            # ---- b = (1-w)^1.5 on [128, 36] (fp32 scalars) ----
            wt = small.tile([128, CO * K * K], f32)
            nc.sync.dma_start(wt[:], w_sc)
            nc.vector.tensor_scalar(wt[:], wt[:], -1.0, 1.0,
                                    Alu.mult, Alu.add)
            sqw = small.tile([128, CO * K * K], f32)
            nc.scalar.activation(sqw[:], wt[:], Act.Sqrt)
            nc.vector.tensor_tensor(wt[:], wt[:], sqw[:], Alu.mult)

            # ---- tropical conv: acc = min over (co,kh,kw) of A_shift+b ----
            acc = big.tile([128, L], bf16)
            acc3 = acc[:].rearrange("p (h w) -> p h w", h=S, w=S)
            rep3 = a_rep[:].rearrange("p (co h w) -> p co h w",
                                      co=CO, h=H, w=H)
            odd3 = a_odd[:].rearrange("p (co h w) -> p co h w",
                                      co=CO, h=H, w=H)
            idx = 0
            for co in range(CO):
                for kh in range(K):
                    for kw in range(K):
                        if kw == 1:
                            av = odd3[:, co, kh:kh + S, 0:S]
                        else:
                            av = rep3[:, co, kh:kh + S, kw:kw + S]
                        bcol = wt[:, co * 9 + kh * 3 + kw:
                                  co * 9 + kh * 3 + kw + 1]
                        if idx == 0:
                            nc.vector.tensor_scalar(acc3, av, bcol, None,
                                                    Alu.add)
                        else:
                            t = tmppool.tile([128, S, S], bf16, tag="t")
                            if idx % 3 == 0:
                                nc.vector.tensor_scalar(t[:], av, bcol,
                                                        None, Alu.add)
                            else:
                                nc.scalar.activation(t[:], av, Act.Identity,
                                                     bias=bcol)
                            nc.vector.tensor_tensor(acc3, acc3, t[:],
                                                    Alu.min)
                        idx += 1

            # ---- reduce over ci: 3 partition-halving min steps ----
            half = big.tile([64, L], bf16)
            for hi, n in ((64, 64), (32, 32), (16, 16)):
                nc.sync.dma_start(half[0:n], acc[hi:hi + n])
                nc.vector.tensor_tensor(acc[0:n], acc[0:n], half[0:n],
                                        Alu.min)

            # ---- out = relu(1 - m^(2/3)) via exp((2/3) ln m) ----
            res = small.tile([FH, L], f32)
            nc.scalar.activation(res[:], acc[0:16], Act.Ln)
            nc.scalar.activation(res[:], res[:], Act.Exp, scale=2.0 / 3.0)
            nc.scalar.activation(res[:], res[:], Act.Relu,
                                 bias=1.0, scale=-1.0)

            nc.sync.dma_start(out_d, res[:])

    nc.compile()
    return nc


def _get_nc():
    if "nc" not in _cache:
        _cache["nc"] = _build_program()
    return _cache["nc"]


def _shard_inputs(x, weight):
    """Host-side sharding/layout. Returns in_maps for 8 cores."""
    in_maps = []
    for core in range(NCORES):
        b = core // 2
        fb = (core % 2) * FH
        xc = np.ascontiguousarray(
            np.asarray(x[b], dtype=np.float32).reshape(XROWS, XCOLS))

        wsl = np.asarray(weight[:, fb:fb + FH], dtype=np.float32)
        # [c, khw, fi] -> [co, ci, khw, fi] -> [ci, fi, co, khw]
        r = wsl.reshape(CO, CI, K * K, FH).transpose(1, 3, 0, 2)
        wsc = np.ascontiguousarray(r).reshape(128, CO * K * K)

        in_maps.append({"x_c": xc, "w_sc": wsc})
    return in_maps


def kernel(x, weight):
    from concourse.bass_utils import run_bass_kernel_spmd

    nc = _get_nc()
    in_maps = _shard_inputs(x, weight)
    res = run_bass_kernel_spmd(nc, in_maps, list(range(NCORES)))
    out = np.empty((B, F, S, S), dtype=np.float32)
    for core in range(NCORES):
        b = core // 2
        fb = (core % 2) * FH
        out[b, fb:fb + FH] = res.results[core]["out"].reshape(FH, S, S)
    return out


# revision 9
# speedup vs baseline: 1.9353x; 1.0702x over previous
"""Trainium2 Bass kernel for nn_MaxYager2d.

Math: out[b,f,sh,sw] = max_j relu(1 - (a_j + b_jf)^(1/p))
  with a_j = (1-xu_j)^p (3x3 unfold windows), b_jf = (1-w_jf)^p, p=1.5.
Since 1-t^(1/p) is monotone decreasing, the max commutes to a min:
  out = relu(1 - (min_j (a_j + b_jf))^(1/p))
i.e. a 3x3 min-plus (tropical) convolution + pointwise pow.

Sharding: 8 cores = 4 batches x 2 halves of F (16 out-channels each).

On-chip layout: 128 partitions = (ci in 0..7) x (fi in 0..15); input
channel c = co*8 + ci with co in 0..3 on the free axis. A = (1-x)^1.5 is
computed compactly, bounced through an HBM scratch, and broadcast-read
16x across the fi partitions. Per (co,kh,kw) slice: tmp = A_shift + b
(4x-mode bf16 tensor_scalar on DVE, or Activation with per-partition
bias on ScalarE -- ACT also absorbs the odd-offset kw=1 slices, which
would drop DVE to 1x), then the 36 tmp slices are min-combined by a mix
of 2x-mode bf16 TT-min on DVE and accumulate-min DMA folds on the
(otherwise idle) DMA engines. A 3-step DMA-realign + TT-min folds the
remaining ci axis; the result is relaid to 128 partitions for a short
ACT chain relu(1 - exp((2/3) ln m)).
"""

import numpy as np

C = 32
K = 3
H = 66          # input spatial
S = 64          # output spatial
L = S * S
F = 32
B = 4
CI = 8          # channels on partitions
CO = 4          # channels on free axis
FH = 16         # f channels per core
HH = H * H      # 4356
XROWS = 128
XCOLS = C * HH // XROWS  # 1089
NCORES = 8

# tuning knobs: per slice-index (0..35) where the add runs and how the
# slice is min-combined.  kw==1 slices (idx%3==1) must use ACT adds.


_cache = {}


def _assignments():
    """Returns add_on_act[36]: which slice-adds run on the Scalar engine."""
    add_on_act = []
    n_extra = 0
    for idx in range(36):
        kw = idx % 3
        if kw == 1:
            add_on_act.append(True)        # 12 unaligned slices
        elif n_extra < 14 and idx % 3 == 0 and idx > 2:
            add_on_act.append(True)        # +14 for load balance => 26
            n_extra += 1
        else:
            add_on_act.append(False)
    return add_on_act


def _build_program():
    import concourse.tile as tile
    from concourse import bacc, mybir

    f32 = mybir.dt.float32
    bf16 = mybir.dt.bfloat16
    Alu = mybir.AluOpType
    Act = mybir.ActivationFunctionType

    add_on_act = _assignments()

    nc = bacc.Bacc("TRN2", target_bir_lowering=False, debug=False,
                   num_devices=NCORES)

    x_c = nc.dram_tensor("x_c", [XROWS, XCOLS], f32,
                         kind="ExternalInput").ap()
    w_sc = nc.dram_tensor("w_sc", [128, CO * K * K], f32,
                          kind="ExternalInput").ap()
    out_d = nc.dram_tensor("out", [FH, L], f32, kind="ExternalOutput").ap()
    a_hbm = nc.dram_tensor("a_scratch", [C * HH], bf16)

    with tile.TileContext(nc) as tc:
        with tc.tile_pool(name="big", bufs=1) as big, \
             tc.tile_pool(name="small", bufs=1) as small, \
             tc.tile_pool(name="tmp", bufs=6) as tmppool:
            # ---- A = (1-x)^1.5 computed compactly ----
            xt = small.tile([XROWS, XCOLS], f32)
            nc.sync.dma_start(xt[:], x_c)
            nc.vector.tensor_scalar(xt[:], xt[:], -1.0, 1.0,
                                    Alu.mult, Alu.add)       # y = 1-x
            sq = small.tile([XROWS, XCOLS], f32)
            nc.scalar.activation(sq[:], xt[:], Act.Sqrt)
            a_c = small.tile([XROWS, XCOLS], bf16)
            nc.vector.tensor_tensor(a_c[:], xt[:], sq[:], Alu.mult)

            # bounce through HBM, broadcast-read 16x across fi partitions
            nc.sync.dma_start(a_hbm.ap(), a_c[:])
            av_ch = a_hbm.ap().rearrange("(c s) -> c s", c=C)  # [32, 4356]
            a_rep = big.tile([128, CO * HH], bf16)
            for co in range(CO):
                blk = av_ch[co * CI:(co + 1) * CI]
                src = blk.unsqueeze(1).broadcast_to([CI, FH, HH])
                nc.sync.dma_start(a_rep[:, co * HH:(co + 1) * HH], src)

            # ---- b = (1-w)^1.5 on [128, 36] (fp32 scalars) ----
            wt = small.tile([128, CO * K * K], f32)
            nc.sync.dma_start(wt[:], w_sc)
            nc.vector.tensor_scalar(wt[:], wt[:], -1.0, 1.0,
                                    Alu.mult, Alu.add)
            sqw = small.tile([128, CO * K * K], f32)
            nc.scalar.activation(sqw[:], wt[:], Act.Sqrt)
            nc.vector.tensor_tensor(wt[:], wt[:], sqw[:], Alu.mult)

            # ---- tropical conv ----
            acc = big.tile([128, L], bf16)
            acc3 = acc[:].rearrange("p (h w) -> p h w", h=S, w=S)
            rep3 = a_rep[:].rearrange("p (co h w) -> p co h w",
                                      co=CO, h=H, w=H)

            def make_add(idx, dest):
                co, kh, kw = idx // 9, (idx // 3) % 3, idx % 3
                av = rep3[:, co, kh:kh + S, kw:kw + S]
                bcol = wt[:, co * 9 + kh * 3 + kw: co * 9 + kh * 3 + kw + 1]
                if add_on_act[idx]:
                    nc.scalar.activation(dest, av, Act.Identity, bias=bcol)
                else:
                    nc.vector.tensor_scalar(dest, av, bcol, None, Alu.add)

            # slice 0 adds straight into acc; the rest produce tmp
            # tiles (ACT or DVE) folded in by the DVE min chain.
            make_add(0, acc3)
            for idx in range(1, 36):
                t = tmppool.tile([128, S, S], bf16, tag="t")
                make_add(idx, t[:])
                nc.vector.tensor_tensor(acc3, acc3, t[:], Alu.min)

            # ---- reduce over ci: 3 partition-halving min steps ----
            half = big.tile([64, L], bf16)
            for hi, n in ((64, 64), (32, 32), (16, 16)):
                nc.sync.dma_start(half[0:n], acc[hi:hi + n])
                nc.vector.tensor_tensor(acc[0:n], acc[0:n], half[0:n],
                                        Alu.min)

            # ---- out = relu(1 - m^(2/3)) via exp((2/3) ln m) ----
            # relay [16, 4096] -> [128, 512] so ACT uses all partitions
            m128 = small.tile([128, L // 8], bf16)
            msrc = acc[0:16].rearrange("p (g r) -> p g r", g=8, r=L // 8)
            nc.sync.dma_start(m128[:, :], msrc)
            res = small.tile([128, L // 8], f32)
            nc.scalar.activation(res[:], m128[:], Act.Ln)
            nc.scalar.activation(res[:], res[:], Act.Exp, scale=2.0 / 3.0)
            nc.scalar.activation(res[:], res[:], Act.Relu,
                                 bias=1.0, scale=-1.0)
            nc.sync.dma_start(
                out_d.rearrange("f (g r) -> f g r", g=8, r=L // 8),
                res[:, :])

    nc.compile()
    return nc


def _get_nc():
    if "nc" not in _cache:
        _cache["nc"] = _build_program()
    return _cache["nc"]


def _shard_inputs(x, weight):
    """Host-side sharding/layout. Returns in_maps for 8 cores."""
    in_maps = []
    for core in range(NCORES):
        b = core // 2
        fb = (core % 2) * FH
        xc = np.ascontiguousarray(
            np.asarray(x[b], dtype=np.float32).reshape(XROWS, XCOLS))

        wsl = np.asarray(weight[:, fb:fb + FH], dtype=np.float32)
        # [c, khw, fi] -> [co, ci, khw, fi] -> [ci, fi, co, khw]
        r = wsl.reshape(CO, CI, K * K, FH).transpose(1, 3, 0, 2)
        wsc = np.ascontiguousarray(r).reshape(128, CO * K * K)

        in_maps.append({"x_c": xc, "w_sc": wsc})
    return in_maps


def kernel(x, weight):
    from concourse.bass_utils import run_bass_kernel_spmd

    nc = _get_nc()
    in_maps = _shard_inputs(x, weight)
    res = run_bass_kernel_spmd(nc, in_maps, list(range(NCORES)))
    out = np.empty((B, F, S, S), dtype=np.float32)
    for core in range(NCORES):
        b = core // 2
        fb = (core % 2) * FH
        out[b, fb:fb + FH] = res.results[core]["out"].reshape(FH, S, S)
    return out


# revision 10
# speedup vs baseline: 1.9778x; 1.0220x over previous
"""Trainium2 Bass kernel for nn_MaxYager2d.

Math: out[b,f,sh,sw] = max_j relu(1 - (a_j + b_jf)^(1/p))
  with a_j = (1-xu_j)^p (3x3 unfold windows), b_jf = (1-w_jf)^p, p=1.5.
Since 1-t^(1/p) is monotone decreasing, the max commutes to a min:
  out = relu(1 - (min_j (a_j + b_jf))^(1/p))
i.e. a 3x3 min-plus (tropical) convolution + pointwise pow.

Sharding: 8 cores = 4 batches x 2 halves of F (16 out-channels each).

On-chip layout: 128 partitions = (ci in 0..7) x (fi in 0..15); input
channel c = co*8 + ci with co in 0..3 on the free axis. A = (1-x)^1.5 is
computed compactly, bounced through an HBM scratch, and broadcast-read
16x across the fi partitions. Per (co,kh,kw) slice: tmp = A_shift + b
(4x-mode bf16 tensor_scalar on DVE, or Activation with per-partition
bias on ScalarE -- ACT also absorbs the odd-offset kw=1 slices, which
would drop DVE to 1x), then the 36 tmp slices are min-combined by a mix
of 2x-mode bf16 TT-min on DVE and accumulate-min DMA folds on the
(otherwise idle) DMA engines. A 3-step DMA-realign + TT-min folds the
remaining ci axis; the result is relaid to 128 partitions for a short
ACT chain relu(1 - exp((2/3) ln m)).
"""

import numpy as np

C = 32
K = 3
H = 66          # input spatial
S = 64          # output spatial
L = S * S
F = 32
B = 4
CI = 8          # channels on partitions
CO = 4          # channels on free axis
FH = 16         # f channels per core
HH = H * H      # 4356
XROWS = 128
XCOLS = C * HH // XROWS  # 1089
NCORES = 8

# tuning knobs: per slice-index (0..35) where the add runs and how the
# slice is min-combined.  kw==1 slices (idx%3==1) must use ACT adds.


_cache = {}


def _assignments():
    """Returns add_on_act[36]: which slice-adds run on the Scalar engine."""
    add_on_act = []
    n_extra = 0
    for idx in range(36):
        kw = idx % 3
        if kw == 1:
            add_on_act.append(True)        # 12 unaligned slices
        elif n_extra < 14 and idx % 3 == 0 and idx > 2:
            add_on_act.append(True)        # +14 for load balance => 26
            n_extra += 1
        else:
            add_on_act.append(False)
    return add_on_act


def finish_half(nc, tc, big, small, acc, out_d, h):
    """ci-reduction + pointwise epilogue on spatial rows [32h, 32h+32)."""
    from concourse import mybir
    f32 = mybir.dt.float32
    bf16 = mybir.dt.bfloat16
    Alu = mybir.AluOpType
    Act = mybir.ActivationFunctionType
    L2 = L // 2
    c0, c1 = h * L2, (h + 1) * L2
    half = big.tile([64, L2], bf16, tag=f"half{h}")
    for hi, n in ((64, 64), (32, 32), (16, 16)):
        nc.sync.dma_start(half[0:n], acc[hi:hi + n, c0:c1])
        nc.vector.tensor_tensor(acc[0:n, c0:c1], acc[0:n, c0:c1],
                                half[0:n], Alu.min)
    # relay [16, 2048] -> [128, 256] so ACT uses all partitions
    m128 = small.tile([128, L2 // 8], bf16, tag=f"m{h}")
    msrc = acc[0:16, c0:c1].rearrange("p (g r) -> p g r", g=8, r=L2 // 8)
    nc.sync.dma_start(m128[:, :], msrc)
    res = small.tile([128, L2 // 8], f32, tag=f"r{h}")
    nc.scalar.activation(res[:], m128[:], Act.Ln)
    nc.scalar.activation(res[:], res[:], Act.Exp, scale=2.0 / 3.0)
    nc.scalar.activation(res[:], res[:], Act.Relu, bias=1.0, scale=-1.0)
    nc.sync.dma_start(
        out_d[:, c0:c1].rearrange("f (g r) -> f g r", g=8, r=L2 // 8),
        res[:, :])


def _build_program():
    import concourse.tile as tile
    from concourse import bacc, mybir

    f32 = mybir.dt.float32
    bf16 = mybir.dt.bfloat16
    Alu = mybir.AluOpType
    Act = mybir.ActivationFunctionType

    add_on_act = _assignments()

    nc = bacc.Bacc("TRN2", target_bir_lowering=False, debug=False,
                   num_devices=NCORES)

    x_c = nc.dram_tensor("x_c", [XROWS, XCOLS], f32,
                         kind="ExternalInput").ap()
    w_sc = nc.dram_tensor("w_sc", [128, CO * K * K], f32,
                          kind="ExternalInput").ap()
    out_d = nc.dram_tensor("out", [FH, L], f32, kind="ExternalOutput").ap()
    a_hbm = nc.dram_tensor("a_scratch", [C * HH], bf16)

    with tile.TileContext(nc) as tc:
        with tc.tile_pool(name="big", bufs=1) as big, \
             tc.tile_pool(name="small", bufs=1) as small, \
             tc.tile_pool(name="tmp", bufs=6) as tmppool:
            # ---- b = (1-w)^1.5 on [128, 36] (fp32 scalars) ----
            wt = small.tile([128, CO * K * K], f32)
            nc.sync.dma_start(wt[:], w_sc)
            nc.vector.tensor_scalar(wt[:], wt[:], -1.0, 1.0,
                                    Alu.mult, Alu.add)
            sqw = small.tile([128, CO * K * K], f32)
            nc.scalar.activation(sqw[:], wt[:], Act.Sqrt)
            nc.vector.tensor_tensor(wt[:], wt[:], sqw[:], Alu.mult)

            # ---- A = (1-x)^1.5, computed compactly in 2 pipelined halves,
            # bounced through HBM and broadcast-read 16x across fi ----
            xt = small.tile([XROWS, XCOLS], f32)
            sq = small.tile([XROWS, XCOLS], f32)
            a_c = small.tile([XROWS, XCOLS], bf16)
            av_ch = a_hbm.ap().rearrange("(c s) -> c s", c=C)  # [32, 4356]
            a_rep = big.tile([128, CO * HH], bf16)
            for h in range(2):
                p0, p1 = 64 * h, 64 * (h + 1)
                nc.sync.dma_start(xt[p0:p1], x_c[p0:p1])
                nc.vector.tensor_scalar(xt[p0:p1], xt[p0:p1], -1.0, 1.0,
                                        Alu.mult, Alu.add)    # y = 1-x
                nc.scalar.activation(sq[p0:p1], xt[p0:p1], Act.Sqrt)
                nc.vector.tensor_tensor(a_c[p0:p1], xt[p0:p1], sq[p0:p1],
                                        Alu.mult)
                nc.sync.dma_start(
                    a_hbm.ap()[h * C * HH // 2:(h + 1) * C * HH // 2],
                    a_c[p0:p1])
                for co in (2 * h, 2 * h + 1):
                    blk = av_ch[co * CI:(co + 1) * CI]
                    src = blk.unsqueeze(1).broadcast_to([CI, FH, HH])
                    nc.sync.dma_start(a_rep[:, co * HH:(co + 1) * HH], src)

            # ---- tropical conv ----
            acc = big.tile([128, L], bf16)
            acc3 = acc[:].rearrange("p (h w) -> p h w", h=S, w=S)
            rep3 = a_rep[:].rearrange("p (co h w) -> p co h w",
                                      co=CO, h=H, w=H)

            def make_add(idx, dest, r0, r1):
                co, kh, kw = idx // 9, (idx // 3) % 3, idx % 3
                av = rep3[:, co, kh + r0:kh + r1, kw:kw + S]
                bcol = wt[:, co * 9 + kh * 3 + kw: co * 9 + kh * 3 + kw + 1]
                if add_on_act[idx]:
                    nc.scalar.activation(dest, av, Act.Identity, bias=bcol)
                else:
                    nc.vector.tensor_scalar(dest, av, bcol, None, Alu.add)

            def chain(idx, r0, r1):
                t = tmppool.tile([128, r1 - r0, S], bf16, tag="t")
                make_add(idx, t[:], r0, r1)
                nc.vector.tensor_tensor(acc3[:, r0:r1], acc3[:, r0:r1],
                                        t[:], Alu.min)

            # slices 0..26 full width (slice 0 adds straight into acc);
            # co=3 slices run split so the top half's tail overlaps the
            # bottom half's chain.
            make_add(0, acc3, 0, S)
            for idx in range(1, 27):
                chain(idx, 0, S)
            for idx in range(27, 36):
                chain(idx, 0, S // 2)
            finish_half(nc, tc, big, small, acc, out_d, 0)
            for idx in range(27, 36):
                chain(idx, S // 2, S)
            finish_half(nc, tc, big, small, acc, out_d, 1)

    nc.compile()
    return nc


def _get_nc():
    if "nc" not in _cache:
        _cache["nc"] = _build_program()
    return _cache["nc"]


def _shard_inputs(x, weight):
    """Host-side sharding/layout. Returns in_maps for 8 cores."""
    in_maps = []
    for core in range(NCORES):
        b = core // 2
        fb = (core % 2) * FH
        xc = np.ascontiguousarray(
            np.asarray(x[b], dtype=np.float32).reshape(XROWS, XCOLS))

        wsl = np.asarray(weight[:, fb:fb + FH], dtype=np.float32)
        # [c, khw, fi] -> [co, ci, khw, fi] -> [ci, fi, co, khw]
        r = wsl.reshape(CO, CI, K * K, FH).transpose(1, 3, 0, 2)
        wsc = np.ascontiguousarray(r).reshape(128, CO * K * K)

        in_maps.append({"x_c": xc, "w_sc": wsc})
    return in_maps


def kernel(x, weight):
    from concourse.bass_utils import run_bass_kernel_spmd

    nc = _get_nc()
    in_maps = _shard_inputs(x, weight)
    res = run_bass_kernel_spmd(nc, in_maps, list(range(NCORES)))
    out = np.empty((B, F, S, S), dtype=np.float32)
    for core in range(NCORES):
        b = core // 2
        fb = (core % 2) * FH
        out[b, fb:fb + FH] = res.results[core]["out"].reshape(FH, S, S)
    return out
